# revision 16
# baseline (speedup 1.0000x reference)
"""Trainium2 Bass kernel for nn_BatchFiGNN (batched graph attention + GRU message passing).

Math (per batch element b, all B=4096 independent):
    attn = softmax(leaky_relu(h@a_src + (h@a_dst)^T, 0.2) * (1-I), axis=-1)
    s = h
    repeat steps(=3):
        gi = attn @ (s @ (w @ W_ih^T)) + (bias @ W_ih^T + b_ih)   # fused W2 = w@W_ih^T
        gh = s @ W_hh^T + b_hh
        r = sig(gi_r + gh_r); z = sig(gi_z + gh_z); n = tanh(gi_n + r*gh_n)
        s = (1-z)*n + z*s + h
    out[b] = (s@mlp2_w + mlp2_b)^T @ (s@mlp1_w + mlp1_b)          # (1,OC)

Distribution: pure data-parallel over batch, B/8 elems per core on 8 cores.
`adj` is unused by the reference math (only its shape) and is never read.

On-chip design (per core):
  - token = (b, n); "pair" = 2 batch elems = 128 tokens = full partition dim.
  - s and h live FEATURE-MAJOR ([128 d, tokens]): weight matmuls take
    lhsT = s_fm chunk directly and every GRU bias folds into ACT per-partition bias.
  - h arrives via dma_start_transpose as a host-split bf16 hi/lo pair and is
    reconstructed to fp32 on-chip (~1e-5 rel err); fp32 DMA transpose is unsupported.
  - message matmul uses block-diagonal attn^T tiles (bf16, K=128); weight matmuls
    run float32r at full rate (free dim >= 256).
  - Phase 1 (exp table set) computes all attention and spills attn^T block-diag
    tiles to DRAM scratch; phase 2 (sigmoid/tanh set) runs GRU steps + MLP head.
    One ACT table switch total.
"""

import sys
from contextlib import ExitStack

sys.path.insert(0, "/opt/trn_rl_repo")

import numpy as np
import ml_dtypes

import concourse.bass as bass
import concourse.bacc as bacc
import concourse.tile as tile
from concourse import mybir
from concourse import bass_utils

FP32 = mybir.dt.float32
FP32R = mybir.dt.float32r
BF16 = mybir.dt.bfloat16

AF = mybir.ActivationFunctionType
ALU = mybir.AluOpType
AX = mybir.AxisListType

N_CORES = 8
N = 64          # nodes per graph
D = 128         # hidden dim
OC = 64         # output channels
G3 = 3 * D      # 384 gate dims


def r32(x):
    return x.bitcast(FP32R)


def build_graph(BL, steps, gp=None):
    """Build + compile the per-core Bass graph for a local batch of BL elements."""
    PAIRS = BL // 2          # 2 batch elems per 128-token pair
    TOK = BL * N             # local tokens
    GP = min(PAIRS, 64) if gp is None else gp      # pairs per group
    GROUPS = PAIRS // GP
    GTOK = GP * 128          # tokens per group
    UNITS = GP // 4          # 4-pair units per group
    CH = min(16, GP)         # pairs per output chunk
    OCH = PAIRS // CH        # output chunks

    nc = bacc.Bacc("TRN2", target_bir_lowering=False, debug=False)

    # ---- DRAM tensors (per-core shard + replicated consts) ----
    h_hi_t = nc.dram_tensor("h_hi", [TOK, D], BF16, kind="ExternalInput")
    h_lo_t = nc.dram_tensor("h_lo", [TOK, D], BF16, kind="ExternalInput")
    w2_t = nc.dram_tensor("w2", [D, G3], FP32R, kind="ExternalInput")       # w @ W_ih^T
    whh_t = nc.dram_tensor("whhT", [D, G3], FP32R, kind="ExternalInput")    # W_hh^T
    actb_t = nc.dram_tensor("actb", [D, 4], FP32, kind="ExternalInput")    # bias cols
    mlp12_t = nc.dram_tensor("mlp12", [D, OC + 2], FP32R, kind="ExternalInput")
    mlpb_t = nc.dram_tensor("mlpb", [D, OC + 2], FP32, kind="ExternalInput")
    asd_t = nc.dram_tensor("asd", [D, 66], BF16, kind="ExternalInput")     # src/dst hi|lo
    ones_t = nc.dram_tensor("ones64", [33, N], BF16, kind="ExternalInput")
    mask_t = nc.dram_tensor("mask4", [128, 4 * N], FP32, kind="ExternalInput")
    ident_t = nc.dram_tensor("identbf", [128, 128], BF16, kind="ExternalInput")
    out_t = nc.dram_tensor("outT", [OCH, OC, 2 * CH], FP32, kind="ExternalOutput")

    with tile.TileContext(nc) as tc, ExitStack() as ctx:
        const = ctx.enter_context(tc.tile_pool(name="const", bufs=1))
        hbuf = ctx.enter_context(tc.tile_pool(name="hbuf", bufs=1))
        sgp = ctx.enter_context(tc.tile_pool(name="sgp", bufs=1))
        work = ctx.enter_context(tc.tile_pool(name="work", bufs=3))
        bdpool = ctx.enter_context(tc.tile_pool(name="bdp", bufs=6))
        gpsum = ctx.enter_context(tc.tile_pool(name="gpsum", bufs=6, space="PSUM"))
        svp = ctx.enter_context(tc.tile_pool(name="svp", bufs=2, space="PSUM"))
        dram = ctx.enter_context(tc.tile_pool(name="dram", bufs=1, space="DRAM"))

        bd_scr = dram.tile([PAIRS, 128, 128], BF16, name="bd_scr")

        # ---- load constants ----
        w2 = const.tile([D, G3], FP32R, name="w2c")
        nc.sync.dma_start(w2, w2_t.ap())
        whh = const.tile([D, G3], FP32R, name="whhc")
        nc.sync.dma_start(whh, whh_t.ap())
        actb = const.tile([D, 4], FP32, name="actbc")
        nc.sync.dma_start(actb, actb_t.ap())
        mlp12 = const.tile([D, OC + 2], FP32R, name="mlp12c")
        nc.sync.dma_start(mlp12, mlp12_t.ap())
        mlpb = const.tile([D, OC + 2], FP32, name="mlpbc")
        nc.sync.dma_start(mlpb, mlpb_t.ap())
        asd = const.tile([D, 66], BF16, name="asdc")
        nc.sync.dma_start(asd, asd_t.ap())
        ones64 = const.tile([33, N], BF16, name="onesc")
        nc.sync.dma_start(ones64, ones_t.ap())
        mask4 = const.tile([128, 4 * N], FP32, name="maskc")
        nc.sync.dma_start(mask4, mask_t.ap())
        identbf = const.tile([128, 128], BF16, name="identc")
        nc.sync.dma_start(identbf, ident_t.ap())

        # ================= PHASE 1: attention =================
        for g in range(GROUPS):
            gtok0 = g * GTOK
            hhi = hbuf.tile([128, GTOK], BF16, tag="hhi", name="hhi1")
            nc.sync.dma_start_transpose(hhi, h_hi_t.ap()[gtok0:gtok0 + GTOK, :])
            hlo = hbuf.tile([128, GTOK], BF16, tag="hlo", name="hlo1")
            nc.sync.dma_start_transpose(hlo, h_lo_t.ap()[gtok0:gtok0 + GTOK, :])

            for u in range(UNITS):
                us = u * 512  # token offset within group
                # src/dst logit rows: src at psum partition 0, dst at partition 32
                sd_ps = gpsum.tile([33, 512], FP32, tag="gp", name="sd_ps")
                nc.tensor.matmul(sd_ps, asd[:, 0:33], hhi[:, us:us + 512],
                                 start=True, stop=False)
                nc.tensor.matmul(sd_ps, asd[:, 0:33], hlo[:, us:us + 512],
                                 start=False, stop=False)
                nc.tensor.matmul(sd_ps, asd[:, 33:66], hhi[:, us:us + 512],
                                 start=False, stop=True)
                sd_sb = work.tile([33, 512], BF16, tag="sdsb", name="sd_sb")
                nc.scalar.activation(sd_sb, sd_ps, AF.Copy)

                # logits for 4 pairs: [128 rows (i of 2 b's), 64 cols (j)] each
                lg_ps = gpsum.tile([128, 4 * N], FP32, tag="gp", name="lg_ps")
                for p in range(4):
                    cs = p * N
                    lt0 = p * 128
                    nc.tensor.matmul(lg_ps[:, cs:cs + N],
                                     sd_sb[0:1, lt0:lt0 + 128], ones64[0:1, :],
                                     start=True, stop=False, skip_group_check=True)
                    nc.tensor.matmul(lg_ps[0:64, cs:cs + N],
                                     ones64[32:33, :], sd_sb[32:33, lt0:lt0 + 64],
                                     start=False, stop=False, skip_group_check=True)
                    nc.tensor.matmul(lg_ps[64:128, cs:cs + N],
                                     ones64[32:33, :], sd_sb[32:33, lt0 + 64:lt0 + 128],
                                     start=False, stop=True, skip_group_check=True)

                # mask first (equivalent: leaky(0)=0), then leaky = max(x, 0.2x)
                l2 = work.tile([128, 4 * N], FP32, tag="l2", name="l2")
                nc.vector.tensor_mul(l2, lg_ps, mask4)
                l3 = work.tile([128, 4 * N], FP32, tag="l3", name="l3")
                nc.vector.scalar_tensor_tensor(l3, l2, 0.2, l2, ALU.mult, ALU.max)
                ee = work.tile([128, 4 * N], FP32, tag="ee", name="ee")
                nc.scalar.activation(ee, l3, AF.Exp)
                rs = work.tile([128, 4], FP32, tag="rs", name="rs")
                nc.vector.tensor_reduce(rs, ee.rearrange("p (q n) -> p q n", n=N),
                                        AX.X, ALU.add)
                rcp = work.tile([128, 4], FP32, tag="rcp", name="rcp")
                nc.vector.reciprocal(rcp, rs)
                an = work.tile([128, 4 * N], BF16, tag="an", name="an")
                nc.vector.tensor_tensor(
                    an.rearrange("p (q n) -> p q n", n=N),
                    ee.rearrange("p (q n) -> p q n", n=N),
                    rcp.rearrange("p (q o) -> p q o", o=1).to_broadcast((128, 4, N)),
                    ALU.mult)

                for p in range(4):
                    pair = g * GP + u * 4 + p
                    tp_ps = svp.tile([64, 128], BF16, tag="sv", name="tp_ps")
                    nc.tensor.transpose(tp_ps, an[:, p * N:(p + 1) * N], identbf)
                    bd = bdpool.tile([128, 128], BF16, tag="bd", name="bd")
                    nc.gpsimd.memset(bd, 0.0)
                    nc.vector.tensor_copy(bd[0:64, 0:64], tp_ps[0:64, 0:64])
                    nc.vector.tensor_copy(bd[64:128, 64:128], tp_ps[0:64, 64:128])
                    nc.sync.dma_start(bd_scr[pair], bd)

        tc.strict_bb_all_engine_barrier()

        # ================= PHASE 2: GRU steps + MLP =================
        for g in range(GROUPS):
            gtok0 = g * GTOK
            hhi = hbuf.tile([128, GTOK], BF16, tag="hhi", name="hhi2")
            nc.sync.dma_start_transpose(hhi, h_hi_t.ap()[gtok0:gtok0 + GTOK, :])
            hlo = hbuf.tile([128, GTOK], BF16, tag="hlo", name="hlo2")
            nc.sync.dma_start_transpose(hlo, h_lo_t.ap()[gtok0:gtok0 + GTOK, :])
            hfm = sgp.tile([128, GTOK], FP32R, tag="hfm", name="hfm")
            nc.vector.tensor_add(hfm, hhi, hlo)
            s_a = sgp.tile([128, GTOK], FP32R, tag="sA", name="s_a")
            s_b = sgp.tile([128, GTOK], FP32R, tag="sB", name="s_b")

            for st in range(steps):
                if st == 0:
                    s_src, s_dst = hfm, s_a
                elif st % 2 == 1:
                    s_src, s_dst = s_a, s_b
                else:
                    s_src, s_dst = s_b, s_a
                for u in range(UNITS):
                    us = u * 512
                    r_ps = gpsum.tile([128, 512], FP32, tag="gp", name="r_ps")
                    z_ps = gpsum.tile([128, 512], FP32, tag="gp", name="z_ps")
                    gi_ps = gpsum.tile([128, 512], FP32, tag="gp", name="gi_ps")
                    gh_ps = gpsum.tile([128, 512], FP32, tag="gp", name="gh_ps")
                    # gh contributions (fp32r, F=512, full rate)
                    nc.tensor.matmul(r_ps, r32(whh[:, 0:128]),
                                     r32(s_src[:, us:us + 512]),
                                     start=True, stop=False, skip_group_check=True)
                    nc.tensor.matmul(z_ps, r32(whh[:, 128:256]),
                                     r32(s_src[:, us:us + 512]),
                                     start=True, stop=False, skip_group_check=True)
                    nc.tensor.matmul(gh_ps, r32(whh[:, 256:384]),
                                     r32(s_src[:, us:us + 512]),
                                     start=True, stop=True)
                    for p in range(4):
                        pair = g * GP + u * 4 + p
                        t0 = us + p * 128
                        cs = p * 128
                        sv_ps = svp.tile([128, G3], FP32, tag="sv", name="sv_ps")
                        nc.tensor.matmul(sv_ps, r32(s_src[:, t0:t0 + 128]),
                                         r32(w2), start=True, stop=True)
                        sv_sb = work.tile([128, G3], BF16, tag="svsb", name="sv_sb")
                        if p % 2 == 0:
                            nc.scalar.activation(sv_sb, sv_ps, AF.Copy)
                        else:
                            nc.vector.tensor_copy(sv_sb, sv_ps)
                        bd = bdpool.tile([128, 128], BF16, tag="bd", name="bd2")
                        nc.sync.dma_start(bd, bd_scr[pair])
                        # message matmuls (bf16): accumulate onto gh for r/z
                        nc.tensor.matmul(r_ps[:, cs:cs + 128], sv_sb[:, 0:128],
                                         bd, start=False, stop=(p == 3),
                                         skip_group_check=True)
                        nc.tensor.matmul(z_ps[:, cs:cs + 128], sv_sb[:, 128:256],
                                         bd, start=False, stop=(p == 3),
                                         skip_group_check=True)
                        nc.tensor.matmul(gi_ps[:, cs:cs + 128], sv_sb[:, 256:384],
                                         bd, start=True, stop=True,
                                         skip_group_check=True)
                    r_sb = work.tile([128, 512], FP32, tag="rsb", name="r_sb")
                    nc.scalar.activation(r_sb, r_ps, AF.Sigmoid, bias=actb[:, 0:1])
                    z_sb = work.tile([128, 512], FP32, tag="zsb", name="z_sb")
                    nc.scalar.activation(z_sb, z_ps, AF.Sigmoid, bias=actb[:, 1:2])
                    rhn = work.tile([128, 512], FP32, tag="rhn", name="rhn")
                    nc.vector.scalar_tensor_tensor(rhn, gh_ps, actb[:, 2:3], r_sb,
                                                   ALU.add, ALU.mult)
                    npre = work.tile([128, 512], FP32, tag="npre", name="npre")
                    nc.vector.tensor_add(npre, gi_ps, rhn)
                    n_sb = work.tile([128, 512], FP32, tag="nsb", name="n_sb")
                    nc.scalar.activation(n_sb, npre, AF.Tanh, bias=actb[:, 3:4])
                    u1 = work.tile([128, 512], FP32, tag="u1", name="u1")
                    nc.gpsimd.tensor_sub(u1, s_src[:, us:us + 512], n_sb)
                    u2 = work.tile([128, 512], FP32, tag="u2", name="u2")
                    nc.vector.tensor_mul(u2, z_sb, u1)
                    u3 = work.tile([128, 512], FP32, tag="u3", name="u3")
                    nc.vector.tensor_add(u3, u2, n_sb)
                    nc.gpsimd.tensor_add(s_dst[:, us:us + 512], u3,
                                         hfm[:, us:us + 512])

            s_fin = s_dst
            # ---- MLP head for this group ----
            for c in range(GP // CH):
                o_ps = gpsum.tile([128, 512], FP32, tag="gp", name="o_ps")
                for q in range(CH):
                    t0 = (c * CH + q) * 128
                    m_ps = svp.tile([128, G3], FP32, tag="sv", name="m_ps")
                    nc.tensor.matmul(m_ps[:, 0:OC + 2], r32(s_fin[:, t0:t0 + 128]),
                                     mlp12, start=True, stop=True,
                                     skip_group_check=True)
                    ow = work.tile([128, OC + 2], FP32, tag="ow", name="ow")
                    nc.vector.tensor_add(ow, m_ps[:, 0:OC + 2], mlpb)
                    nc.tensor.matmul(o_ps[0:OC, 2 * q:2 * q + 1],
                                     ow[0:64, 0:OC], ow[0:64, OC:OC + 1],
                                     start=True, stop=True, skip_group_check=True)
                    nc.tensor.matmul(o_ps[0:OC, 2 * q + 1:2 * q + 2],
                                     ow[64:128, 0:OC], ow[64:128, OC:OC + 1],
                                     start=True, stop=True, skip_group_check=True)
                ost = work.tile([OC, 2 * CH], FP32, tag="ost", name="ost")
                nc.vector.tensor_copy(ost, o_ps[0:OC, 0:2 * CH])
                nc.sync.dma_start(out_t.ap()[g * (GP // CH) + c], ost)

    nc.compile()
    return nc


_CACHE = {}


def _get_graph(BL, steps):
    key = (BL, steps)
    if key not in _CACHE:
        _CACHE[key] = build_graph(BL, steps)
    return _CACHE[key]


def host_prep(a_src, a_dst, w, bias, W_ih, W_hh, b_ih, b_hh,
              mlp1_w, mlp1_b, mlp2_w, mlp2_b):
    """Shared (replicated) device inputs."""
    f32 = np.float32
    bf16 = ml_dtypes.bfloat16

    w2 = (np.asarray(w, f32) @ np.asarray(W_ih, f32).T).astype(f32)        # (D, 3D)
    c2 = (np.asarray(bias, f32) @ np.asarray(W_ih, f32).T
          + np.asarray(b_ih, f32)).astype(f32)                              # (3D,)
    whhT = np.ascontiguousarray(np.asarray(W_hh, f32).T)                    # (D, 3D)
    b_hh = np.asarray(b_hh, f32)

    actb = np.stack([c2[0:D] + b_hh[0:D],
                     c2[D:2 * D] + b_hh[D:2 * D],
                     b_hh[2 * D:3 * D],
                     c2[2 * D:3 * D]], axis=1).astype(f32)                  # (D, 4)

    mlp12 = np.concatenate([np.asarray(mlp1_w, f32),
                            np.asarray(mlp2_w, f32),
                            np.zeros((D, 1), f32)], axis=1)                 # (D, 66)
    mlpb = np.broadcast_to(
        np.concatenate([np.asarray(mlp1_b, f32),
                        np.asarray(mlp2_b, f32),
                        np.zeros((1,), f32)])[None, :], (D, OC + 2)
    ).copy()

    a_sd = np.concatenate([np.asarray(a_src, f32),
                           np.asarray(a_dst, f32)], axis=1)                 # (D, 2)
    a_hi = a_sd.astype(bf16)
    a_lo = (a_sd - a_hi.astype(f32)).astype(bf16)
    # widened lhsT: src in col 0, dst in col 32 (-> psum partitions 0 / 32)
    asd = np.zeros((D, 66), dtype=bf16)
    asd[:, 0] = a_hi[:, 0]
    asd[:, 32] = a_hi[:, 1]
    asd[:, 33] = a_lo[:, 0]
    asd[:, 65] = a_lo[:, 1]

    ones64 = np.ones((33, N), dtype=bf16)
    mpair = np.concatenate([1.0 - np.eye(N, dtype=f32)] * 2, axis=0)        # (128,64)
    mask4 = np.tile(mpair, (1, 4)).astype(f32)                              # (128,256)
    identbf = np.eye(128, dtype=bf16)

    return dict(w2=w2, whhT=whhT, actb=actb, mlp12=mlp12, mlpb=mlpb,
                asd=asd, ones64=ones64, mask4=mask4, identbf=identbf)


def make_in_maps(h, shared, n_cores=N_CORES):
    f32 = np.float32
    bf16 = ml_dtypes.bfloat16
    B = h.shape[0]
    BL = B // n_cores
    in_maps = []
    for c in range(n_cores):
        hl = np.asarray(h[c * BL:(c + 1) * BL], f32).reshape(BL * N, D)
        h_hi = hl.astype(bf16)
        h_lo = (hl - h_hi.astype(f32)).astype(bf16)
        m = dict(shared)
        m["h_hi"] = h_hi
        m["h_lo"] = h_lo
        in_maps.append(m)
    return in_maps


def assemble_out(raw_outs, BL):
    """raw_outs: per-core 'outT' arrays (OCH, OC, 2*CH) -> (B, OC)."""
    outs = [np.ascontiguousarray(o.transpose(0, 2, 1)).reshape(BL, OC)
            for o in raw_outs]
    return np.concatenate(outs, axis=0).astype(np.float32)


def run(h, adj, a_src, a_dst, w, bias, W_ih, W_hh, b_ih, b_hh,
        mlp1_w, mlp1_b, mlp2_w, mlp2_b, steps, trace=False, **run_kwargs):
    h = np.asarray(h, np.float32)
    B = h.shape[0]
    BL = B // N_CORES
    steps = int(steps)

    nc = _get_graph(BL, steps)
    shared = host_prep(a_src, a_dst, w, bias, W_ih, W_hh, b_ih, b_hh,
                       mlp1_w, mlp1_b, mlp2_w, mlp2_b)
    in_maps = make_in_maps(h, shared)

    res = bass_utils.run_bass_kernel_spmd(nc, in_maps, core_ids=list(range(N_CORES)),
                                          trace=trace, **run_kwargs)
    out = assemble_out([res.results[c]["outT"] for c in range(N_CORES)], BL)
    return out, res


def kernel(**inputs):
    out, _ = run(**inputs)
    return out
